# revision 20
# baseline (speedup 1.0000x reference)
"""Trainium2 Bass kernel for nn_DecoderLSTM (30-step decoder LSTM, npeds=8192,
hidden=256, embed=64), data-parallel over peds across 8 NeuronCores.

v2: fp8(e4m3) DoubleRow matmuls for the recurrent gates.

Layout (per core, 1024 peds in 2 pipelined halves of 512):
  - Transposed: partitions = feature dims, free = peds. States are
    pair-packed [128, 2(ch), 512]: ch0 = h[0:128], ch1 = h[128:256].
  - Gates: one fp8 DoubleRow matmul contracts all 256 h rows per
    128-row gate chunk (pair dim = ch), plus one DR matmul for the
    dec/embedding contribution, whose moving tensor is a [66, 512]
    fp8 tile broadcast into both pair slots (stride-0). Pair slot 1 of
    the dec stationary carries the W_ih fp8 RESIDUAL (free accuracy),
    and rows 64/65 are constant 1.0 whose stationary entries encode the
    gate bias as 4 fp8 terms (hi+lo on each row) -> bias ~exact.
  - The tanh gate (g) gets one extra DR matmul with the W_hh fp8
    residual (g is the precision-critical gate).
  - Weights are pre-scaled by S (pow2); sigma/tanh activations unscale
    via the ACT scale operand. With the bias in the matmul, sigmoid of
    (i,f,o) for one ch runs as a single fused [128,3*512] activation
    over 3 adjacent PSUM banks.
  - LayerNorm2 folded as in v1: stats rows (A0 h, A1 h, mean, E[h^2])
    by PE matmuls; E[h^2] via an fp8-DR matmul against h8*h8 (Pool).
    Tail per-ped math in a 32x32 block-transposed domain, with custom
    DVE ops (VFUSE, RSQ_NR newton, PRELU) crushing the op count.
  - h kept in bf16 (stats path) and fp8 (gate path); Pool produces the
    fp8 copy and its square; DVE produces the bf16 copy.
  - Outputs are DMA'd in the compact tail domain [32, 32] per
    (step, half) and un-interleaved on the host.
"""
import os
import sys

for _p in ("/root/.axon_site/_ro/trn_rl_repo", "/opt/trn_rl_repo"):
    if os.path.isdir(_p) and _p not in sys.path:
        sys.path.insert(0, _p)

import numpy as np
import ml_dtypes

import concourse.bass as bass
import concourse.tile as tile
from concourse import bacc, mybir
from concourse import bass_utils
from concourse.bass_interp import get_hw_module


def _ensure_ntff_hook_module():
    """Provide antenv.axon_hooks if the image ships without it, so
    run_bass_kernel_spmd(trace=True) can capture NTFF profiles."""
    try:
        from antenv import axon_hooks  # noqa: F401
        return
    except ImportError:
        pass
    import types

    mod = types.ModuleType("antenv.axon_hooks")
    mod._HOOK = None

    def set_axon_ntff_profile_hook(hook):
        mod._HOOK = hook

    def get_axon_ntff_profile_hook():
        if mod._HOOK is None:
            try:
                from trn_agent_boot.trn_boot import _ntff_profile_via_ctypes
                mod._HOOK = _ntff_profile_via_ctypes("/opt/axon/libaxon_pjrt.so")
            except Exception:
                mod._HOOK = None
        return mod._HOOK

    mod.set_axon_ntff_profile_hook = set_axon_ntff_profile_hook
    mod.get_axon_ntff_profile_hook = get_axon_ntff_profile_hook
    sys.modules["antenv.axon_hooks"] = mod
    try:
        import antenv
        antenv.axon_hooks = mod
    except ImportError:
        pass


_ensure_ntff_hook_module()

# ---- custom DVE ops (registered into concourse.dve_ops) ----
from concourse import dve_ops as dvo
from concourse.dve_spec import Spec, Src0, Src1, C0, C1, C2, lower, relu
from concourse.dve_uop import DveOpSpec
from concourse.dve_ops import DveOp, OPS, CUSTOM_DVE_SPECS


def _register_op(name, body, ref):
    if name in CUSTOM_DVE_SPECS:
        return next(op for op in OPS if op.name == name)
    spec = Spec(body=body, reference=ref)
    shas = {}
    for ver in ("v3", "v4"):
        s = DveOpSpec(name=name, opcode=0, uops=lower(spec, ver=ver),
                      rd1_en=dvo.has_src1(spec))
        shas[ver] = s.sha(ver)
    op = DveOp(name, spec, subdim=False, uops_sha=shas)
    OPS.append(op)
    CUSTOM_DVE_SPECS[name] = spec
    dvo._SUB_OPCODE_FOR_NAME[name] = max(dvo._SUB_OPCODE_FOR_NAME.values()) + 1
    return op


# out = (x + b)*c1 + relu(x + b)*c2   (prelu with per-partition bias+scale)
PRELU_OP = _register_op(
    "ANT_PRELU_BIAS", (Src0 + C0) * C1 + relu(Src0 + C0) * C2,
    lambda in0, in1, s0, s1, imm2: (in0 + s0) * s1 + np.maximum(in0 + s0, 0) * imm2)
# out = (x*c1 + c0) - y*y     (variance from E[h^2], mu, +eps)
VFUSE_OP = _register_op(
    "ANT_VFUSE", (Src0 * C1 + C0) - Src1 * Src1,
    lambda in0, in1, s0, s1, imm2: (in0 * s1 + s0) - in1 * in1)
# out = y*(c0 - (v*y)*y*c1)   (one rsqrt newton step; v=Src0, y=Src1)
RSQNR_OP = _register_op(
    "ANT_RSQ_NR", Src1 * (C0 - (Src0 * Src1) * Src1 * C1),
    lambda in0, in1, s0, s1, imm2: in1 * (s0 - (in0 * in1) * in1 * s1))

F32 = mybir.dt.float32
BF16 = mybir.dt.bfloat16
FP8 = mybir.dt.float8e4
I32 = mybir.dt.int32
AF = mybir.ActivationFunctionType
OP = mybir.AluOpType
DR = mybir.MatmulPerfMode.DoubleRow

E4NP = ml_dtypes.float8_e4m3
BFNP = ml_dtypes.bfloat16

N_CORES = 8
NPEDS = 8192
NP_CORE = NPEDS // N_CORES      # 1024
HALF = NP_CORE // 2             # 512
H = 256
E = 64
T = 30
EPS = 1e-5
LEAK = 0.01
MAGIC = 0x5F3759DF
KAP = 240.0                     # ones-row magnitude for bias terms
TANH_S_SCALE = 0.88 / float(np.sqrt(4.0 * EPS))

# gate chunk order along stationary free axis: ch-major, (i,f,o,g)
GATE_BASE = {"i": 0, "f": H, "g": 2 * H, "o": 3 * H}
CHUNKS = [(g, ch) for ch in range(2) for g in ("i", "f", "o", "g")]


def _chunk_rows(gate, ch):
    b = GATE_BASE[gate] + ch * 128
    return slice(b, b + 128)


DEBUG_TAPS = False


def _build_program():
    nc = bacc.Bacc(
        "TRN2",
        target_bir_lowering=False,
        debug=False,
        enable_asserts=False,
        num_devices=N_CORES,
    )

    d = {}
    d["LWH"] = nc.dram_tensor("LWH", [128, 8, 2, 128], FP8, kind="ExternalInput")
    d["LWHD"] = nc.dram_tensor("LWHD", [128, 2, 2, 128], FP8, kind="ExternalInput")
    d["LWD"] = nc.dram_tensor("LWD", [66, 8, 2, 128], FP8, kind="ExternalInput")
    d["AS"] = nc.dram_tensor("AS", [128, 2, 32], BF16, kind="ExternalInput")
    d["SQDR"] = nc.dram_tensor("SQDR", [128, 2, 32], FP8, kind="ExternalInput")
    d["WEMB"] = nc.dram_tensor("WEMB", [1, 64], BF16, kind="ExternalInput")
    d["EMBB"] = nc.dram_tensor("EMBB", [64, 1], F32, kind="ExternalInput")
    d["PB0"] = nc.dram_tensor("PB0", [32, 1], F32, kind="ExternalInput")
    d["PB1"] = nc.dram_tensor("PB1", [32, 1], F32, kind="ExternalInput")
    d["H0"] = nc.dram_tensor("H0", [128, 2, NP_CORE], BF16, kind="ExternalInput")
    d["H08"] = nc.dram_tensor("H08", [128, 2, NP_CORE], FP8, kind="ExternalInput")
    d["C0"] = nc.dram_tensor("C0", [128, 2, NP_CORE], BF16, kind="ExternalInput")
    d["DEC0"] = nc.dram_tensor("DEC0", [66, NP_CORE], FP8, kind="ExternalInput")
    out_t = nc.dram_tensor("OUT", [T, 2, 32, 32], F32, kind="ExternalOutput")
    dbg = {}
    if DEBUG_TAPS:
        for nm, shp in [("dSG", [128, 2, 3, HALF]), ("dTG", [128, 2, HALF]),
                        ("dCS", [128, 2, HALF]), ("dTB", [128, 2, HALF]),
                        ("dH8", [128, 2, HALF]), ("dHSQ", [128, 2, HALF]),
                        ("dST", [32, HALF]), ("dTT", [32, HALF]),
                        ("dV", [32, 16]), ("dR", [32, 16]),
                        ("dDEC", [66, HALF])]:
            dbg[nm] = nc.dram_tensor(nm, shp, F32, kind="ExternalOutput")

    inv_s = None  # patched below once S is known at prep time; use attr
    # activation scale 1/S is a compile-time constant: S is fixed pow2
    # computed from the spec'd weight magnitudes; recompute here must match
    # _prepare_in_maps. We hardcode S=512 (see _prep: weights ~N(0,0.05),
    # max|W| in (0.109, 0.4375] -> S=512).
    S = 512.0
    INV_S = 1.0 / S

    with tile.TileContext(nc) as tc:
        with (
            tc.tile_pool(name="weights", bufs=1) as wp,
            tc.tile_pool(name="state", bufs=1) as sp,
            tc.tile_pool(name="acts", bufs=2) as ap_,
            tc.tile_pool(name="dve", bufs=4) as dp,
            tc.tile_pool(name="tail", bufs=2) as tp,
            tc.tile_pool(name="tappool", bufs=1) as tapp,
            tc.tile_pool(name="pifo", bufs=2, space="PSUM") as pifo,
            tc.tile_pool(name="psm", bufs=2, space="PSUM") as psm,
        ):
            # ---- persistent weights ----
            LWH = [wp.tile([128, 2, 128], FP8, name=f"LWH{c}", tag=f"LWH{c}") for c in range(8)]
            LWHD = [wp.tile([128, 2, 128], FP8, name=f"LWHD{c}", tag=f"LWHD{c}") for c in range(2)]
            LWD = [wp.tile([66, 2, 128], FP8, name=f"LWD{c}", tag=f"LWD{c}") for c in range(8)]
            AS = wp.tile([128, 2, 32], BF16, tag="AS")
            SQDR = wp.tile([128, 2, 32], FP8, tag="SQDR")
            WEMB = wp.tile([1, 64], BF16, tag="WEMB")
            EMBB = wp.tile([64, 1], F32, tag="EMBB")
            PB0 = wp.tile([32, 1], F32, tag="PB0")
            PB1 = wp.tile([32, 1], F32, tag="PB1")
            IONE = wp.tile([32, 16], I32, tag="IONE")
            IMAG = wp.tile([32, 16], I32, tag="IMAG")
            for c in range(8):
                nc.sync.dma_start(LWH[c][:], d["LWH"].ap()[:, c])
                nc.sync.dma_start(LWD[c][:], d["LWD"].ap()[:, c])
            for c in range(2):
                nc.sync.dma_start(LWHD[c][:], d["LWHD"].ap()[:, c])
            for nm, t_ in [("AS", AS), ("SQDR", SQDR), ("WEMB", WEMB),
                           ("EMBB", EMBB), ("PB0", PB0), ("PB1", PB1)]:
                nc.sync.dma_start(t_[:], d[nm].ap())
            nc.vector.memset(IONE[:], 1)
            nc.vector.memset(IMAG[:], MAGIC)

            # ---- persistent states [half][parity] ----
            TB = [[sp.tile([128, 2, HALF], BF16, name=f"TB{h}{p}", tag=f"TB{h}{p}")
                   for p in range(2)] for h in range(2)]
            H8 = [[sp.tile([128, 2, HALF], FP8, name=f"H8{h}{p}", tag=f"H8{h}{p}")
                   for p in range(2)] for h in range(2)]
            CS = [[sp.tile([128, 2, HALF], BF16, name=f"CS{h}{p}", tag=f"CS{h}{p}")
                   for p in range(2)] for h in range(2)]
            DEC8 = [[sp.tile([66, HALF], FP8, name=f"DEC{h}{p}", tag=f"DEC{h}{p}")
                     for p in range(2)] for h in range(2)]
            SSB = [sp.tile([32, HALF], BF16, name=f"SSB{h}", tag=f"SSB{h}") for h in range(2)]
            for h in range(2):
                cols = slice(h * HALF, (h + 1) * HALF)
                nc.sync.dma_start(TB[h][0][:], d["H0"].ap()[:, :, cols])
                nc.sync.dma_start(H8[h][0][:], d["H08"].ap()[:, :, cols])
                nc.sync.dma_start(CS[h][0][:], d["C0"].ap()[:, :, cols])
                nc.sync.dma_start(DEC8[h][0][:], d["DEC0"].ap()[:, cols])
                nc.vector.memset(DEC8[h][1][64:66, :], 1.0)
                nc.vector.memset(SSB[h][:], 0.0)

            def tap(nm, src, shp):
                if not DEBUG_TAPS:
                    return
                tt_ = tapp.tile(shp, F32, name=f"tap{nm}", tag=f"tap{nm}")
                nc.scalar.activation(tt_[:], src, AF.Copy)
                nc.sync.dma_start(dbg[nm].ap(), tt_[:])

            def gates_and_cell(h, p, q, dotap=False):
                """Gate matmuls + activations + cell update for one half."""
                sg = ap_.tile([128, 2, 3, HALF], BF16, name=f"SG{h}", tag=f"SG{h}")
                tg = ap_.tile([128, 2, HALF], BF16, name=f"TG{h}", tag=f"TG{h}")
                h8 = H8[h][p][:]
                dec = DEC8[h][p][:].unsqueeze(1).broadcast_to([66, 2, HALF])
                for ch in range(2):
                    wave = pifo.tile([128, 3, HALF], F32, tag="ifo")
                    for gi, gate in enumerate(("i", "f", "o")):
                        c = ch * 4 + {"i": 0, "f": 1, "o": 2, "g": 3}[gate]
                        o_ = wave[:, gi, :]
                        nc.tensor.matmul(o_, LWH[c][:], h8, start=True,
                                         stop=False, perf_mode=DR)
                        nc.tensor.matmul(o_, LWD[c][:], dec, start=False,
                                         stop=True, perf_mode=DR)
                    nc.scalar.activation(sg[:, ch, :, :], wave[:], AF.Sigmoid,
                                         scale=INV_S)
                    gp = psm.tile([128, HALF], F32, tag="sm")
                    c = ch * 4 + 3
                    nc.tensor.matmul(gp[:], LWH[c][:], h8, start=True,
                                     stop=False, perf_mode=DR)
                    nc.tensor.matmul(gp[:], LWHD[ch][:], h8, start=False,
                                     stop=False, perf_mode=DR)
                    nc.tensor.matmul(gp[:], LWD[c][:], dec, start=False,
                                     stop=True, perf_mode=DR)
                    nc.scalar.activation(tg[:, ch, :], gp[:], AF.Tanh,
                                         scale=INV_S)
                sg_i = sg[:, :, 0, :]
                sg_f = sg[:, :, 1, :]
                sg_o = sg[:, :, 2, :]
                m1 = dp.tile([128, 2, HALF], BF16, tag="m1")
                nc.vector.tensor_tensor(m1[:], sg_f, CS[h][p][:], OP.mult)
                m2 = dp.tile([128, 2, HALF], BF16, tag="m2")
                nc.vector.tensor_tensor(m2[:], sg_i, tg[:], OP.mult)
                cn = CS[h][q]
                nc.vector.tensor_tensor(cn[:], m1[:], m2[:], OP.add)
                tc_ = ap_.tile([128, 2, HALF], BF16, name=f"TC{h}", tag=f"TC{h}")
                nc.scalar.activation(tc_[:], cn[:], AF.Tanh)
                nc.vector.tensor_tensor(TB[h][q][:], sg_o, tc_[:], OP.mult)
                # fp8 h copy + its square on Pool (off DVE critical path)
                nc.gpsimd.tensor_tensor(H8[h][q][:], sg_o, tc_[:], OP.mult)
                hsq = dp.tile([128, 2, HALF], FP8, name=f"HSQ{h}", tag=f"HSQ{h}")
                nc.gpsimd.tensor_tensor(hsq[:], H8[h][q][:], H8[h][q][:],
                                        OP.mult)
                if dotap:
                    tap("dSG", sg[:], [128, 2, 3, HALF])
                    tap("dTG", tg[:], [128, 2, HALF])
                    tap("dCS", cn[:], [128, 2, HALF])
                    tap("dTB", TB[h][q][:], [128, 2, HALF])
                    tap("dH8", H8[h][q][:], [128, 2, HALF])
                    tap("dHSQ", hsq[:], [128, 2, HALF])
                return hsq

            def stats(h, q, hsq):
                stt = psm.tile([128, HALF], F32, tag="sm")
                st = stt[0:32, :]
                nc.tensor.matmul(st, AS[:, 0, :], TB[h][q][:, 0, :],
                                 start=True, stop=False)
                nc.tensor.matmul(st, AS[:, 1, :], TB[h][q][:, 1, :],
                                 start=False, stop=False)
                nc.tensor.matmul(st, SQDR[:], hsq[:], start=False, stop=True,
                                 perf_mode=DR)
                return st

            def tail(t_, h, st, dotap=False):
                if dotap:
                    tap("dST", st, [32, HALF])
                tailT = tp.tile([32, HALF], F32, tag="tailT")
                nc.vector.transpose(tailT[:], st)
                if dotap:
                    tap("dTT", tailT[:], [32, HALF])
                c_num0 = tailT[:, 0::32]
                c_num1 = tailT[:, 1::32]
                c_mu = tailT[:, 2::32]
                c_eh2 = tailT[:, 3::32]
                V = dp.tile([32, 16], F32, tag="V")
                nc.vector._custom_dve(VFUSE_OP, out=V[:], in0=c_eh2,
                                      in1=c_mu, s0=EPS, s1=1.0)
                sh = dp.tile([32, 16], I32, tag="sh")
                nc.vector.tensor_tensor(sh[:], V[:].bitcast(I32), IONE[:],
                                        OP.arith_shift_right)
                y = dp.tile([32, 16], F32, tag="y")
                nc.vector.tensor_tensor(y[:].bitcast(I32), IMAG[:], sh[:],
                                        OP.subtract)
                r = dp.tile([32, 16], F32, tag="r")
                nc.vector._custom_dve(RSQNR_OP, out=r[:], in0=V[:], in1=y[:],
                                      s0=1.5, s1=0.5)
                if dotap:
                    tap("dV", V[:], [32, 16])
                    tap("dR", r[:], [32, 16])
                z0 = dp.tile([32, 16], F32, tag="z0")
                nc.vector.tensor_tensor(z0[:], c_num0, r[:], OP.mult)
                z1 = dp.tile([32, 16], F32, tag="z1")
                nc.vector.tensor_tensor(z1[:], c_num1, r[:], OP.mult)
                ts = tp.tile([32, 32], F32, tag="TS")
                nc.scalar.activation(ts[:, 0:16], z0[:], AF.Sigmoid,
                                     bias=PB0[:])
                nc.scalar.activation(ts[:, 16:32], z1[:], AF.Sigmoid,
                                     bias=PB1[:])
                e = dp.tile([32, 16], F32, tag="e")
                nc.vector.tensor_tensor(e[:], ts[:, 0:16], ts[:, 16:32],
                                        OP.subtract)
                nc.scalar.activation(SSB[h][:, 0::32], e[:], AF.Tanh,
                                     scale=TANH_S_SCALE)
                nc.sync.dma_start(out_t.ap()[t_][h], ts[:])
                sB = tp.tile([32, HALF], BF16, tag="sB")
                nc.vector.transpose(sB[:], SSB[h][:])
                return sB

            def embed(h, q, sB, dotap=False):
                pet = psm.tile([128, HALF], F32, tag="sm")
                pe = pet[0:64, :]
                nc.tensor.matmul(pe, WEMB[:], sB[0:1, :], start=True,
                                 stop=True)
                nc.vector._custom_dve(PRELU_OP, out=DEC8[h][q][0:64, :],
                                      in0=pe, s0=EMBB[:], s1=LEAK, imm2=1.0 - LEAK)
                if dotap:
                    tap("dDEC", DEC8[h][q][:], [66, HALF])

            for t_ in range(T):
                p, q = t_ % 2, (t_ + 1) % 2
                dt0 = DEBUG_TAPS and t_ == 0
                hsq0 = gates_and_cell(0, p, q, dotap=dt0)
                hsq1 = gates_and_cell(1, p, q)
                st0 = stats(0, q, hsq0)
                st1 = stats(1, q, hsq1)
                sB0 = tail(t_, 0, st0, dotap=dt0)
                sB1 = tail(t_, 1, st1)
                embed(0, q, sB0, dotap=dt0)
                embed(1, q, sB1)

    nc.compile()
    return nc


_NC_CACHE = None


def _get_program():
    global _NC_CACHE
    if _NC_CACHE is None:
        _NC_CACHE = _build_program()
    return _NC_CACHE


def _prepare_in_maps(inputs):
    f32 = np.float32
    inp = {k: np.asarray(v, f32) for k, v in inputs.items()}
    W_ih, W_hh = inp["W_ih"], inp["W_hh"]
    bias = (inp["b_ih"] + inp["b_hh"]).astype(f32)

    S = 2.0 ** np.floor(np.log2(224.0 / max(np.abs(W_hh).max(),
                                            np.abs(W_ih).max())))
    assert S == 512.0, f"S={S} changed; update INV_S in _build_program"

    def q8(x):
        return np.asarray(x, E4NP).astype(f32)

    Whh8 = q8(W_hh * S)
    Wih8 = q8(W_ih * S)
    dWhh8 = q8(W_hh * S - Whh8)
    dWih8 = q8(W_ih * S - Wih8)
    bhi = q8(bias * S)
    blo = q8(bias * S - bhi)
    r1 = bias * S - (bhi + blo)
    bhi2 = q8(r1)
    blo2 = q8(r1 - bhi2)

    LWH = np.zeros((128, 8, 2, 128), f32)
    LWHD = np.zeros((128, 2, 2, 128), f32)
    LWD = np.zeros((66, 8, 2, 128), f32)
    for ci, (gate, ch) in enumerate(CHUNKS):
        rows = _chunk_rows(gate, ch)
        for j in range(2):
            LWH[:, ci, j, :] = Whh8[rows, 128 * j:128 * (j + 1)].T
            LWD[0:64, ci, j, :] = (Wih8 if j == 0 else dWih8)[rows].T
        LWD[64, ci, 0, :] = bhi[rows]
        LWD[64, ci, 1, :] = blo[rows]
        LWD[65, ci, 0, :] = bhi2[rows]
        LWD[65, ci, 1, :] = blo2[rows]
    for ch in range(2):
        rows = _chunk_rows("g", ch)
        for j in range(2):
            LWHD[:, ch, j, :] = dWhh8[rows, 128 * j:128 * (j + 1)].T

    emb_W, emb_b = inp["emb_W"], inp["emb_b"]
    g1, b1 = inp["ln1_g"], inp["ln1_b"]
    w_emb = (g1[0] * emb_W[:, 0] - g1[1] * emb_W[:, 1]).astype(f32)
    emb_bp = (emb_b + b1[0] * emb_W[:, 0] + b1[1] * emb_W[:, 1]).astype(f32)
    WEMB = w_emb[None, :].astype(f32)

    pos_W, pos_b = inp["pos_W"], inp["pos_b"]
    g2, b2 = inp["ln2_g"], inp["ln2_b"]
    posWp = (pos_W * g2[None, :]).astype(f32)
    pos_bp = (pos_b + b2 @ pos_W.T).astype(f32)
    A = posWp - posWp.sum(1)[:, None] / H

    ASm = np.zeros((128, 2, 32), f32)
    for ch in range(2):
        ASm[:, ch, 0] = A[0, 128 * ch:128 * (ch + 1)]
        ASm[:, ch, 1] = A[1, 128 * ch:128 * (ch + 1)]
        ASm[:, ch, 2] = 1.0 / H
    SQDR = np.zeros((128, 2, 32), f32)
    SQDR[:, :, 3] = 1.0 / H

    lpr = inp["last_pos_rel"]
    e0 = lpr[:, 0] - lpr[:, 1]
    s0 = e0 / np.sqrt(e0 * e0 + 4 * EPS)
    z = s0[:, None] * w_emb[None, :] + emb_bp[None, :]
    dec0 = np.where(z > 0, z, LEAK * z).astype(f32)       # [N, 64]
    DEC0 = np.ones((66, NPEDS), f32)
    DEC0[0:64] = dec0.T

    h0 = inp["h0"][0]                                      # [N, 256]
    c0 = inp["c0"][0]
    h0p = np.ascontiguousarray(h0.T.reshape(2, 128, NPEDS).transpose(1, 0, 2))
    c0p = np.ascontiguousarray(c0.T.reshape(2, 128, NPEDS).transpose(1, 0, 2))

    rep = {
        "LWH": LWH.astype(E4NP), "LWHD": LWHD.astype(E4NP),
        "LWD": LWD.astype(E4NP),
        "AS": ASm.astype(BFNP), "SQDR": SQDR.astype(E4NP),
        "WEMB": WEMB.astype(BFNP),
        "EMBB": np.ascontiguousarray(emb_bp[:, None]),
        "PB0": np.full((32, 1), pos_bp[0], f32),
        "PB1": np.full((32, 1), pos_bp[1], f32),
    }
    in_maps = []
    for c in range(N_CORES):
        cols = slice(c * NP_CORE, (c + 1) * NP_CORE)
        m = dict(rep)
        m["H0"] = np.ascontiguousarray(h0p[:, :, cols]).astype(BFNP)
        m["H08"] = np.ascontiguousarray(h0p[:, :, cols]).astype(E4NP)
        m["C0"] = np.ascontiguousarray(c0p[:, :, cols]).astype(BFNP)
        m["DEC0"] = np.ascontiguousarray(DEC0[:, cols]).astype(E4NP)
        in_maps.append(m)
    return in_maps


def _unscramble(raw):
    """raw: [T, 2, 32, 32] per core -> [T, NP_CORE, 2]."""
    o = raw.reshape(T, 2, 32, 2, 16)          # t, half, p, j, k
    o = o.transpose(0, 1, 4, 2, 3)            # t, half, k, p, j
    return o.reshape(T, NP_CORE, 2)


def run_on_hw(inputs, trace=False, **kwargs):
    nc = _get_program()
    in_maps = _prepare_in_maps(inputs)
    old_m = nc.m
    nc.m = get_hw_module(nc.m)
    try:
        res = bass_utils.run_bass_kernel_spmd(
            nc, in_maps, core_ids=list(range(N_CORES)), trace=trace, **kwargs)
    finally:
        nc.m = old_m
    out = np.concatenate([_unscramble(np.asarray(r["OUT"], np.float32))
                          for r in res.results], axis=1)
    return out.astype(np.float32), res


def kernel(**inputs) -> np.ndarray:
    out, _ = run_on_hw(inputs, trace=False)
    return out


# revision 22
# speedup vs baseline: 1.1187x; 1.1187x over previous
"""Trainium2 Bass kernel for nn_DecoderLSTM (30-step decoder LSTM, npeds=8192,
hidden=256, embed=64), data-parallel over peds across 8 NeuronCores.

v2: fp8(e4m3) DoubleRow matmuls for the recurrent gates.

Layout (per core, 1024 peds in 2 pipelined halves of 512):
  - Transposed: partitions = feature dims, free = peds. States are
    pair-packed [128, 2(ch), 512]: ch0 = h[0:128], ch1 = h[128:256].
  - Gates: one fp8 DoubleRow matmul contracts all 256 h rows per
    128-row gate chunk (pair dim = ch), plus one DR matmul for the
    dec/embedding contribution, whose moving tensor is a [66, 512]
    fp8 tile broadcast into both pair slots (stride-0). Pair slot 1 of
    the dec stationary carries the W_ih fp8 RESIDUAL (free accuracy),
    and rows 64/65 are constant 1.0 whose stationary entries encode the
    gate bias as 4 fp8 terms (hi+lo on each row) -> bias ~exact.
  - The tanh gate (g) gets one extra DR matmul with the W_hh fp8
    residual (g is the precision-critical gate).
  - Weights are pre-scaled by S (pow2); sigma/tanh activations unscale
    via the ACT scale operand. With the bias in the matmul, sigmoid of
    (i,f,o) for one ch runs as a single fused [128,3*512] activation
    over 3 adjacent PSUM banks.
  - LayerNorm2 folded as in v1: stats rows (A0 h, A1 h, mean, E[h^2])
    by PE matmuls; E[h^2] via an fp8-DR matmul against h8*h8 (Pool).
    Tail per-ped math in a 32x32 block-transposed domain, with custom
    DVE ops (VFUSE, RSQ_NR newton, PRELU) crushing the op count.
  - h kept in bf16 (stats path) and fp8 (gate path); Pool produces the
    fp8 copy and its square; DVE produces the bf16 copy.
  - Outputs are DMA'd in the compact tail domain [32, 32] per
    (step, half) and un-interleaved on the host.
"""
import os
import sys

for _p in ("/root/.axon_site/_ro/trn_rl_repo", "/opt/trn_rl_repo"):
    if os.path.isdir(_p) and _p not in sys.path:
        sys.path.insert(0, _p)

import numpy as np
import ml_dtypes

import concourse.bass as bass
import concourse.tile as tile
from concourse import bacc, mybir
from concourse import bass_utils
from concourse.bass_interp import get_hw_module


def _ensure_ntff_hook_module():
    """Provide antenv.axon_hooks if the image ships without it, so
    run_bass_kernel_spmd(trace=True) can capture NTFF profiles."""
    try:
        from antenv import axon_hooks  # noqa: F401
        return
    except ImportError:
        pass
    import types

    mod = types.ModuleType("antenv.axon_hooks")
    mod._HOOK = None

    def set_axon_ntff_profile_hook(hook):
        mod._HOOK = hook

    def get_axon_ntff_profile_hook():
        if mod._HOOK is None:
            try:
                from trn_agent_boot.trn_boot import _ntff_profile_via_ctypes
                mod._HOOK = _ntff_profile_via_ctypes("/opt/axon/libaxon_pjrt.so")
            except Exception:
                mod._HOOK = None
        return mod._HOOK

    mod.set_axon_ntff_profile_hook = set_axon_ntff_profile_hook
    mod.get_axon_ntff_profile_hook = get_axon_ntff_profile_hook
    sys.modules["antenv.axon_hooks"] = mod
    try:
        import antenv
        antenv.axon_hooks = mod
    except ImportError:
        pass


_ensure_ntff_hook_module()

# ---- custom DVE ops (registered into concourse.dve_ops) ----
from concourse import dve_ops as dvo
from concourse.dve_spec import Spec, Src0, Src1, C0, C1, C2, lower, relu
from concourse.dve_uop import DveOpSpec
from concourse.dve_ops import DveOp, OPS, CUSTOM_DVE_SPECS


def _register_op(name, body, ref):
    if name in CUSTOM_DVE_SPECS:
        return next(op for op in OPS if op.name == name)
    spec = Spec(body=body, reference=ref)
    shas = {}
    for ver in ("v3", "v4"):
        s = DveOpSpec(name=name, opcode=0, uops=lower(spec, ver=ver),
                      rd1_en=dvo.has_src1(spec))
        shas[ver] = s.sha(ver)
    op = DveOp(name, spec, subdim=False, uops_sha=shas)
    OPS.append(op)
    CUSTOM_DVE_SPECS[name] = spec
    dvo._SUB_OPCODE_FOR_NAME[name] = max(dvo._SUB_OPCODE_FOR_NAME.values()) + 1
    return op


# out = (x + b)*c1 + relu(x + b)*c2   (prelu with per-partition bias+scale)
PRELU_OP = _register_op(
    "ANT_PRELU_BIAS", (Src0 + C0) * C1 + relu(Src0 + C0) * C2,
    lambda in0, in1, s0, s1, imm2: (in0 + s0) * s1 + np.maximum(in0 + s0, 0) * imm2)
# out = (x*c1 + c0) - y*y     (variance from E[h^2], mu, +eps)
VFUSE_OP = _register_op(
    "ANT_VFUSE", (Src0 * C1 + C0) - Src1 * Src1,
    lambda in0, in1, s0, s1, imm2: (in0 * s1 + s0) - in1 * in1)
# out = y*(c0 - (v*y)*y*c1)   (one rsqrt newton step; v=Src0, y=Src1)
RSQNR_OP = _register_op(
    "ANT_RSQ_NR", Src1 * (C0 - (Src0 * Src1) * Src1 * C1),
    lambda in0, in1, s0, s1, imm2: in1 * (s0 - (in0 * in1) * in1 * s1))

F32 = mybir.dt.float32
BF16 = mybir.dt.bfloat16
FP8 = mybir.dt.float8e4
I32 = mybir.dt.int32
AF = mybir.ActivationFunctionType
OP = mybir.AluOpType
DR = mybir.MatmulPerfMode.DoubleRow

E4NP = ml_dtypes.float8_e4m3
BFNP = ml_dtypes.bfloat16

N_CORES = 8
NPEDS = 8192
NP_CORE = NPEDS // N_CORES      # 1024
HALF = NP_CORE // 2             # 512
H = 256
E = 64
T = 30
EPS = 1e-5
LEAK = 0.01
MAGIC = 0x5F3759DF
KAP = 240.0                     # ones-row magnitude for bias terms
TANH_S_SCALE = 0.88 / float(np.sqrt(4.0 * EPS))

# gate chunk order along stationary free axis: ch-major, (i,f,o,g)
GATE_BASE = {"i": 0, "f": H, "g": 2 * H, "o": 3 * H}
CHUNKS = [(g, ch) for ch in range(2) for g in ("i", "f", "o", "g")]


def _chunk_rows(gate, ch):
    b = GATE_BASE[gate] + ch * 128
    return slice(b, b + 128)


DEBUG_TAPS = False


def _build_program():
    nc = bacc.Bacc(
        "TRN2",
        target_bir_lowering=False,
        debug=False,
        enable_asserts=False,
        num_devices=N_CORES,
    )

    d = {}
    d["LWH"] = nc.dram_tensor("LWH", [128, 8, 2, 128], FP8, kind="ExternalInput")
    d["LWHD"] = nc.dram_tensor("LWHD", [128, 2, 2, 128], FP8, kind="ExternalInput")
    d["LWD"] = nc.dram_tensor("LWD", [66, 8, 2, 128], FP8, kind="ExternalInput")
    d["AS"] = nc.dram_tensor("AS", [128, 2, 32], BF16, kind="ExternalInput")
    d["SQB"] = nc.dram_tensor("SQB", [128, 2, 32], BF16, kind="ExternalInput")
    d["WEMB"] = nc.dram_tensor("WEMB", [1, 64], BF16, kind="ExternalInput")
    d["EMBB"] = nc.dram_tensor("EMBB", [64, 1], F32, kind="ExternalInput")
    d["PB0"] = nc.dram_tensor("PB0", [32, 1], F32, kind="ExternalInput")
    d["PB1"] = nc.dram_tensor("PB1", [32, 1], F32, kind="ExternalInput")
    d["H0"] = nc.dram_tensor("H0", [128, 2, NP_CORE], BF16, kind="ExternalInput")
    d["H08"] = nc.dram_tensor("H08", [128, 2, NP_CORE], FP8, kind="ExternalInput")
    d["C0"] = nc.dram_tensor("C0", [128, 2, NP_CORE], BF16, kind="ExternalInput")
    d["DEC0"] = nc.dram_tensor("DEC0", [66, NP_CORE], FP8, kind="ExternalInput")
    out_t = nc.dram_tensor("OUT", [T, 2, 32, 32], F32, kind="ExternalOutput")
    dbg = {}
    if DEBUG_TAPS:
        for nm, shp in [("dSG", [128, 2, 3, HALF]), ("dTG", [128, 2, HALF]),
                        ("dCS", [128, 2, HALF]), ("dTB", [128, 2, HALF]),
                        ("dH8", [128, 2, HALF]), ("dHSQ", [128, 2, HALF]),
                        ("dST", [32, HALF]), ("dTT", [32, HALF]),
                        ("dV", [32, 16]), ("dR", [32, 16]),
                        ("dDEC", [66, HALF])]:
            dbg[nm] = nc.dram_tensor(nm, shp, F32, kind="ExternalOutput")

    inv_s = None  # patched below once S is known at prep time; use attr
    # activation scale 1/S is a compile-time constant: S is fixed pow2
    # computed from the spec'd weight magnitudes; recompute here must match
    # _prepare_in_maps. We hardcode S=512 (see _prep: weights ~N(0,0.05),
    # max|W| in (0.109, 0.4375] -> S=512).
    S = 512.0
    INV_S = 1.0 / S

    with tile.TileContext(nc) as tc:
        with (
            tc.tile_pool(name="weights", bufs=1) as wp,
            tc.tile_pool(name="state", bufs=1) as sp,
            tc.tile_pool(name="acts", bufs=2) as ap_,
            tc.tile_pool(name="dve", bufs=4) as dp,
            tc.tile_pool(name="tail", bufs=2) as tp,
            tc.tile_pool(name="tappool", bufs=1) as tapp,
            tc.tile_pool(name="pifo", bufs=2, space="PSUM") as pifo,
            tc.tile_pool(name="psm", bufs=2, space="PSUM") as psm,
        ):
            # ---- persistent weights ----
            LWH = [wp.tile([128, 2, 128], FP8, name=f"LWH{c}", tag=f"LWH{c}") for c in range(8)]
            LWHD = [wp.tile([128, 2, 128], FP8, name=f"LWHD{c}", tag=f"LWHD{c}") for c in range(2)]
            LWD = [wp.tile([66, 2, 128], FP8, name=f"LWD{c}", tag=f"LWD{c}") for c in range(8)]
            AS = wp.tile([128, 2, 32], BF16, tag="AS")
            SQB = wp.tile([128, 2, 32], BF16, tag="SQB")
            WEMB = wp.tile([1, 64], BF16, tag="WEMB")
            EMBB = wp.tile([64, 1], F32, tag="EMBB")
            PB0 = wp.tile([32, 1], F32, tag="PB0")
            PB1 = wp.tile([32, 1], F32, tag="PB1")
            IONE = wp.tile([32, 16], I32, tag="IONE")
            IMAG = wp.tile([32, 16], I32, tag="IMAG")
            for c in range(8):
                nc.sync.dma_start(LWH[c][:], d["LWH"].ap()[:, c])
                nc.sync.dma_start(LWD[c][:], d["LWD"].ap()[:, c])
            for c in range(2):
                nc.sync.dma_start(LWHD[c][:], d["LWHD"].ap()[:, c])
            for nm, t_ in [("AS", AS), ("SQB", SQB), ("WEMB", WEMB),
                           ("EMBB", EMBB), ("PB0", PB0), ("PB1", PB1)]:
                nc.sync.dma_start(t_[:], d[nm].ap())
            nc.vector.memset(IONE[:], 1)
            nc.vector.memset(IMAG[:], MAGIC)

            # ---- persistent states [half][parity] ----
            TB = [[sp.tile([128, 2, HALF], BF16, name=f"TB{h}{p}", tag=f"TB{h}{p}")
                   for p in range(2)] for h in range(2)]
            H8 = [[sp.tile([128, 2, HALF], FP8, name=f"H8{h}{p}", tag=f"H8{h}{p}")
                   for p in range(2)] for h in range(2)]
            CS = [[sp.tile([128, 2, HALF], BF16, name=f"CS{h}{p}", tag=f"CS{h}{p}")
                   for p in range(2)] for h in range(2)]
            DEC8 = [[sp.tile([66, HALF], FP8, name=f"DEC{h}{p}", tag=f"DEC{h}{p}")
                     for p in range(2)] for h in range(2)]
            SSB = [sp.tile([32, HALF], BF16, name=f"SSB{h}", tag=f"SSB{h}") for h in range(2)]
            for h in range(2):
                cols = slice(h * HALF, (h + 1) * HALF)
                nc.sync.dma_start(TB[h][0][:], d["H0"].ap()[:, :, cols])
                nc.sync.dma_start(H8[h][0][:], d["H08"].ap()[:, :, cols])
                nc.sync.dma_start(CS[h][0][:], d["C0"].ap()[:, :, cols])
                nc.sync.dma_start(DEC8[h][0][:], d["DEC0"].ap()[:, cols])
                nc.vector.memset(DEC8[h][1][64:66, :], 1.0)
                nc.vector.memset(SSB[h][:], 0.0)

            def tap(nm, src, shp):
                if not DEBUG_TAPS:
                    return
                tt_ = tapp.tile(shp, F32, name=f"tap{nm}", tag=f"tap{nm}")
                nc.scalar.activation(tt_[:], src, AF.Copy)
                nc.sync.dma_start(dbg[nm].ap(), tt_[:])

            def g_mms(h, p):
                """All gate matmuls for one half; returns psum pieces."""
                h8 = H8[h][p][:]
                dec = DEC8[h][p][:].unsqueeze(1).broadcast_to([66, 2, HALF])
                waves = []
                for ch in range(2):
                    wave = pifo.tile([128, 3, HALF], F32, tag="ifo")
                    for gi, gate in enumerate(("i", "f", "o")):
                        c = ch * 4 + {"i": 0, "f": 1, "o": 2}[gate]
                        o_ = wave[:, gi, :]
                        nc.tensor.matmul(o_, LWH[c][:], h8, start=True,
                                         stop=False, perf_mode=DR)
                        nc.tensor.matmul(o_, LWD[c][:], dec, start=False,
                                         stop=True, perf_mode=DR)
                    waves.append(wave)
                gps = []
                for ch in range(2):
                    gp = psm.tile([128, HALF], F32, tag="sm")
                    c = ch * 4 + 3
                    nc.tensor.matmul(gp[:], LWH[c][:], h8, start=True,
                                     stop=False, perf_mode=DR)
                    nc.tensor.matmul(gp[:], LWHD[ch][:], h8, start=False,
                                     stop=False, perf_mode=DR)
                    nc.tensor.matmul(gp[:], LWD[c][:], dec, start=False,
                                     stop=True, perf_mode=DR)
                    gps.append(gp)
                sg = ap_.tile([128, 2, 3, HALF], BF16, name=f"SG{h}",
                              tag=f"SG{h}")
                tg = ap_.tile([128, 2, HALF], BF16, name=f"TG{h}",
                              tag=f"TG{h}")
                return {"waves": waves, "gps": gps, "sg": sg, "tg": tg}

            def act_sig(h, G, ch):
                nc.scalar.activation(G["sg"][:, ch, :, :], G["waves"][ch][:],
                                     AF.Sigmoid, scale=INV_S)

            def act_tg(h, G, ch):
                nc.scalar.activation(G["tg"][:, ch, :], G["gps"][ch][:],
                                     AF.Tanh, scale=INV_S)

            def cell_dve(h, p, q, G):
                sg, tg = G["sg"], G["tg"]
                m1 = dp.tile([128, 2, HALF], BF16, tag="m1")
                nc.vector.tensor_tensor(m1[:], sg[:, :, 1, :], CS[h][p][:],
                                        OP.mult)
                m2 = dp.tile([128, 2, HALF], BF16, tag="m2")
                nc.vector.tensor_tensor(m2[:], sg[:, :, 0, :], tg[:], OP.mult)
                nc.vector.tensor_tensor(CS[h][q][:], m1[:], m2[:], OP.add)

            def act_tc(h, q, G):
                tc_ = ap_.tile([128, 2, HALF], BF16, name=f"TC{h}",
                               tag=f"TC{h}")
                nc.scalar.activation(tc_[:], CS[h][q][:], AF.Tanh)
                G["tc"] = tc_

            def hn_dve(h, q, G):
                sg_o = G["sg"][:, :, 2, :]
                tc_ = G["tc"]
                nc.vector.tensor_tensor(TB[h][q][:], sg_o, tc_[:], OP.mult)
                hsq = dp.tile([128, 2, HALF], BF16, name=f"HSQ{h}",
                              tag=f"HSQ{h}")
                nc.vector.tensor_tensor(hsq[:], TB[h][q][:], TB[h][q][:],
                                        OP.mult)
                # fp8 h copy on Pool (needed only by next step's gate mms)
                nc.gpsimd.tensor_tensor(H8[h][q][:], sg_o, tc_[:], OP.mult)
                return hsq

            def stats(h, q, hsq):
                stt = psm.tile([128, HALF], F32, tag="sm")
                st = stt[0:32, :]
                nc.tensor.matmul(st, AS[:, 0, :], TB[h][q][:, 0, :],
                                 start=True, stop=False)
                nc.tensor.matmul(st, AS[:, 1, :], TB[h][q][:, 1, :],
                                 start=False, stop=False)
                nc.tensor.matmul(st, SQB[:, 0, :], hsq[:, 0, :],
                                 start=False, stop=False)
                nc.tensor.matmul(st, SQB[:, 1, :], hsq[:, 1, :],
                                 start=False, stop=True)
                return st

            def tail(t_, h, st, dotap=False):
                if dotap:
                    tap("dST", st, [32, HALF])
                tailT = tp.tile([32, HALF], F32, tag="tailT")
                nc.vector.transpose(tailT[:], st)
                if dotap:
                    tap("dTT", tailT[:], [32, HALF])
                c_num0 = tailT[:, 0::32]
                c_num1 = tailT[:, 1::32]
                c_mu = tailT[:, 2::32]
                c_eh2 = tailT[:, 3::32]
                V = dp.tile([32, 16], F32, tag="V")
                nc.vector._custom_dve(VFUSE_OP, out=V[:], in0=c_eh2,
                                      in1=c_mu, s0=EPS, s1=1.0)
                sh = dp.tile([32, 16], I32, tag="sh")
                nc.vector.tensor_tensor(sh[:], V[:].bitcast(I32), IONE[:],
                                        OP.arith_shift_right)
                y = dp.tile([32, 16], F32, tag="y")
                nc.vector.tensor_tensor(y[:].bitcast(I32), IMAG[:], sh[:],
                                        OP.subtract)
                r = dp.tile([32, 16], F32, tag="r")
                nc.vector._custom_dve(RSQNR_OP, out=r[:], in0=V[:], in1=y[:],
                                      s0=1.5, s1=0.5)
                if dotap:
                    tap("dV", V[:], [32, 16])
                    tap("dR", r[:], [32, 16])
                z0 = dp.tile([32, 16], F32, tag="z0")
                nc.vector.tensor_tensor(z0[:], c_num0, r[:], OP.mult)
                z1 = dp.tile([32, 16], F32, tag="z1")
                nc.vector.tensor_tensor(z1[:], c_num1, r[:], OP.mult)
                ts = tp.tile([32, 32], F32, tag="TS")
                nc.scalar.activation(ts[:, 0:16], z0[:], AF.Sigmoid,
                                     bias=PB0[:])
                nc.scalar.activation(ts[:, 16:32], z1[:], AF.Sigmoid,
                                     bias=PB1[:])
                e = dp.tile([32, 16], F32, tag="e")
                nc.vector.tensor_tensor(e[:], ts[:, 0:16], ts[:, 16:32],
                                        OP.subtract)
                nc.scalar.activation(SSB[h][:, 0::32], e[:], AF.Tanh,
                                     scale=TANH_S_SCALE)
                nc.sync.dma_start(out_t.ap()[t_][h], ts[:])
                sB = tp.tile([32, HALF], BF16, tag="sB")
                nc.vector.transpose(sB[:], SSB[h][:])
                return sB

            def embed(h, q, sB, dotap=False):
                pet = psm.tile([128, HALF], F32, tag="sm")
                pe = pet[0:64, :]
                nc.tensor.matmul(pe, WEMB[:], sB[0:1, :], start=True,
                                 stop=True)
                nc.vector._custom_dve(PRELU_OP, out=DEC8[h][q][0:64, :],
                                      in0=pe, s0=EMBB[:], s1=LEAK, imm2=1.0 - LEAK)
                if dotap:
                    tap("dDEC", DEC8[h][q][:], [66, HALF])

            for t_ in range(T):
                p, q = t_ % 2, (t_ + 1) % 2
                dt0 = DEBUG_TAPS and t_ == 0
                G0 = g_mms(0, p)
                G1 = g_mms(1, p)
                act_sig(0, G0, 0)
                act_sig(0, G0, 1)
                act_tg(0, G0, 0)
                act_tg(0, G0, 1)
                cell_dve(0, p, q, G0)
                act_sig(1, G1, 0)
                act_tc(0, q, G0)
                act_sig(1, G1, 1)
                hsq0 = hn_dve(0, q, G0)
                act_tg(1, G1, 0)
                act_tg(1, G1, 1)
                cell_dve(1, p, q, G1)
                act_tc(1, q, G1)
                hsq1 = hn_dve(1, q, G1)
                st0 = stats(0, q, hsq0)
                st1 = stats(1, q, hsq1)
                sB0 = tail(t_, 0, st0, dotap=dt0)
                sB1 = tail(t_, 1, st1)
                embed(0, q, sB0, dotap=dt0)
                embed(1, q, sB1)

    nc.compile()
    return nc


_NC_CACHE = None


def _get_program():
    global _NC_CACHE
    if _NC_CACHE is None:
        _NC_CACHE = _build_program()
    return _NC_CACHE


def _prepare_in_maps(inputs):
    f32 = np.float32
    inp = {k: np.asarray(v, f32) for k, v in inputs.items()}
    W_ih, W_hh = inp["W_ih"], inp["W_hh"]
    bias = (inp["b_ih"] + inp["b_hh"]).astype(f32)

    S = 2.0 ** np.floor(np.log2(224.0 / max(np.abs(W_hh).max(),
                                            np.abs(W_ih).max())))
    assert S == 512.0, f"S={S} changed; update INV_S in _build_program"

    def q8(x):
        return np.asarray(x, E4NP).astype(f32)

    Whh8 = q8(W_hh * S)
    Wih8 = q8(W_ih * S)
    dWhh8 = q8(W_hh * S - Whh8)
    dWih8 = q8(W_ih * S - Wih8)
    bhi = q8(bias * S)
    blo = q8(bias * S - bhi)
    r1 = bias * S - (bhi + blo)
    bhi2 = q8(r1)
    blo2 = q8(r1 - bhi2)

    LWH = np.zeros((128, 8, 2, 128), f32)
    LWHD = np.zeros((128, 2, 2, 128), f32)
    LWD = np.zeros((66, 8, 2, 128), f32)
    for ci, (gate, ch) in enumerate(CHUNKS):
        rows = _chunk_rows(gate, ch)
        for j in range(2):
            LWH[:, ci, j, :] = Whh8[rows, 128 * j:128 * (j + 1)].T
            LWD[0:64, ci, j, :] = (Wih8 if j == 0 else dWih8)[rows].T
        LWD[64, ci, 0, :] = bhi[rows]
        LWD[64, ci, 1, :] = blo[rows]
        LWD[65, ci, 0, :] = bhi2[rows]
        LWD[65, ci, 1, :] = blo2[rows]
    for ch in range(2):
        rows = _chunk_rows("g", ch)
        for j in range(2):
            LWHD[:, ch, j, :] = dWhh8[rows, 128 * j:128 * (j + 1)].T

    emb_W, emb_b = inp["emb_W"], inp["emb_b"]
    g1, b1 = inp["ln1_g"], inp["ln1_b"]
    w_emb = (g1[0] * emb_W[:, 0] - g1[1] * emb_W[:, 1]).astype(f32)
    emb_bp = (emb_b + b1[0] * emb_W[:, 0] + b1[1] * emb_W[:, 1]).astype(f32)
    WEMB = w_emb[None, :].astype(f32)

    pos_W, pos_b = inp["pos_W"], inp["pos_b"]
    g2, b2 = inp["ln2_g"], inp["ln2_b"]
    posWp = (pos_W * g2[None, :]).astype(f32)
    pos_bp = (pos_b + b2 @ pos_W.T).astype(f32)
    A = posWp - posWp.sum(1)[:, None] / H

    ASm = np.zeros((128, 2, 32), f32)
    for ch in range(2):
        ASm[:, ch, 0] = A[0, 128 * ch:128 * (ch + 1)]
        ASm[:, ch, 1] = A[1, 128 * ch:128 * (ch + 1)]
        ASm[:, ch, 2] = 1.0 / H
    SQB = np.zeros((128, 2, 32), f32)
    SQB[:, :, 3] = 1.0 / H

    lpr = inp["last_pos_rel"]
    e0 = lpr[:, 0] - lpr[:, 1]
    s0 = e0 / np.sqrt(e0 * e0 + 4 * EPS)
    z = s0[:, None] * w_emb[None, :] + emb_bp[None, :]
    dec0 = np.where(z > 0, z, LEAK * z).astype(f32)       # [N, 64]
    DEC0 = np.ones((66, NPEDS), f32)
    DEC0[0:64] = dec0.T

    h0 = inp["h0"][0]                                      # [N, 256]
    c0 = inp["c0"][0]
    h0p = np.ascontiguousarray(h0.T.reshape(2, 128, NPEDS).transpose(1, 0, 2))
    c0p = np.ascontiguousarray(c0.T.reshape(2, 128, NPEDS).transpose(1, 0, 2))

    rep = {
        "LWH": LWH.astype(E4NP), "LWHD": LWHD.astype(E4NP),
        "LWD": LWD.astype(E4NP),
        "AS": ASm.astype(BFNP), "SQB": SQB.astype(BFNP),
        "WEMB": WEMB.astype(BFNP),
        "EMBB": np.ascontiguousarray(emb_bp[:, None]),
        "PB0": np.full((32, 1), pos_bp[0], f32),
        "PB1": np.full((32, 1), pos_bp[1], f32),
    }
    in_maps = []
    for c in range(N_CORES):
        cols = slice(c * NP_CORE, (c + 1) * NP_CORE)
        m = dict(rep)
        m["H0"] = np.ascontiguousarray(h0p[:, :, cols]).astype(BFNP)
        m["H08"] = np.ascontiguousarray(h0p[:, :, cols]).astype(E4NP)
        m["C0"] = np.ascontiguousarray(c0p[:, :, cols]).astype(BFNP)
        m["DEC0"] = np.ascontiguousarray(DEC0[:, cols]).astype(E4NP)
        in_maps.append(m)
    return in_maps


def _unscramble(raw):
    """raw: [T, 2, 32, 32] per core -> [T, NP_CORE, 2]."""
    o = raw.reshape(T, 2, 32, 2, 16)          # t, half, p, j, k
    o = o.transpose(0, 1, 4, 2, 3)            # t, half, k, p, j
    return o.reshape(T, NP_CORE, 2)


def run_on_hw(inputs, trace=False, **kwargs):
    nc = _get_program()
    in_maps = _prepare_in_maps(inputs)
    old_m = nc.m
    nc.m = get_hw_module(nc.m)
    try:
        res = bass_utils.run_bass_kernel_spmd(
            nc, in_maps, core_ids=list(range(N_CORES)), trace=trace, **kwargs)
    finally:
        nc.m = old_m
    out = np.concatenate([_unscramble(np.asarray(r["OUT"], np.float32))
                          for r in res.results], axis=1)
    return out.astype(np.float32), res


def kernel(**inputs) -> np.ndarray:
    out, _ = run_on_hw(inputs, trace=False)
    return out


# revision 23
# speedup vs baseline: 1.1650x; 1.0414x over previous
"""Trainium2 Bass kernel for nn_DecoderLSTM (30-step decoder LSTM, npeds=8192,
hidden=256, embed=64), data-parallel over peds across 8 NeuronCores.

v2: fp8(e4m3) DoubleRow matmuls for the recurrent gates.

Layout (per core, 1024 peds in 2 pipelined halves of 512):
  - Transposed: partitions = feature dims, free = peds. States are
    pair-packed [128, 2(ch), 512]: ch0 = h[0:128], ch1 = h[128:256].
  - Gates: one fp8 DoubleRow matmul contracts all 256 h rows per
    128-row gate chunk (pair dim = ch), plus one DR matmul for the
    dec/embedding contribution, whose moving tensor is a [66, 512]
    fp8 tile broadcast into both pair slots (stride-0). Pair slot 1 of
    the dec stationary carries the W_ih fp8 RESIDUAL (free accuracy),
    and rows 64/65 are constant 1.0 whose stationary entries encode the
    gate bias as 4 fp8 terms (hi+lo on each row) -> bias ~exact.
  - The tanh gate (g) gets one extra DR matmul with the W_hh fp8
    residual (g is the precision-critical gate).
  - Weights are pre-scaled by S (pow2); sigma/tanh activations unscale
    via the ACT scale operand. With the bias in the matmul, sigmoid of
    (i,f,o) for one ch runs as a single fused [128,3*512] activation
    over 3 adjacent PSUM banks.
  - LayerNorm2 folded as in v1: stats rows (A0 h, A1 h, mean, E[h^2])
    by PE matmuls; E[h^2] via an fp8-DR matmul against h8*h8 (Pool).
    Tail per-ped math in a 32x32 block-transposed domain, with custom
    DVE ops (VFUSE, RSQ_NR newton, PRELU) crushing the op count.
  - h kept in bf16 (stats path) and fp8 (gate path); Pool produces the
    fp8 copy and its square; DVE produces the bf16 copy.
  - Outputs are DMA'd in the compact tail domain [32, 32] per
    (step, half) and un-interleaved on the host.
"""
import os
import sys

for _p in ("/root/.axon_site/_ro/trn_rl_repo", "/opt/trn_rl_repo"):
    if os.path.isdir(_p) and _p not in sys.path:
        sys.path.insert(0, _p)

import numpy as np
import ml_dtypes

import concourse.bass as bass
import concourse.tile as tile
from concourse import bacc, mybir
from concourse import bass_utils
from concourse.bass_interp import get_hw_module


def _ensure_ntff_hook_module():
    """Provide antenv.axon_hooks if the image ships without it, so
    run_bass_kernel_spmd(trace=True) can capture NTFF profiles."""
    try:
        from antenv import axon_hooks  # noqa: F401
        return
    except ImportError:
        pass
    import types

    mod = types.ModuleType("antenv.axon_hooks")
    mod._HOOK = None

    def set_axon_ntff_profile_hook(hook):
        mod._HOOK = hook

    def get_axon_ntff_profile_hook():
        if mod._HOOK is None:
            try:
                from trn_agent_boot.trn_boot import _ntff_profile_via_ctypes
                mod._HOOK = _ntff_profile_via_ctypes("/opt/axon/libaxon_pjrt.so")
            except Exception:
                mod._HOOK = None
        return mod._HOOK

    mod.set_axon_ntff_profile_hook = set_axon_ntff_profile_hook
    mod.get_axon_ntff_profile_hook = get_axon_ntff_profile_hook
    sys.modules["antenv.axon_hooks"] = mod
    try:
        import antenv
        antenv.axon_hooks = mod
    except ImportError:
        pass


_ensure_ntff_hook_module()

# ---- custom DVE ops (registered into concourse.dve_ops) ----
from concourse import dve_ops as dvo
from concourse.dve_spec import Spec, Src0, Src1, C0, C1, C2, lower, relu
from concourse.dve_uop import DveOpSpec
from concourse.dve_ops import DveOp, OPS, CUSTOM_DVE_SPECS


def _register_op(name, body, ref):
    if name in CUSTOM_DVE_SPECS:
        return next(op for op in OPS if op.name == name)
    spec = Spec(body=body, reference=ref)
    shas = {}
    for ver in ("v3", "v4"):
        s = DveOpSpec(name=name, opcode=0, uops=lower(spec, ver=ver),
                      rd1_en=dvo.has_src1(spec))
        shas[ver] = s.sha(ver)
    op = DveOp(name, spec, subdim=False, uops_sha=shas)
    OPS.append(op)
    CUSTOM_DVE_SPECS[name] = spec
    dvo._SUB_OPCODE_FOR_NAME[name] = max(dvo._SUB_OPCODE_FOR_NAME.values()) + 1
    return op


# out = (x + b)*c1 + relu(x + b)*c2   (prelu with per-partition bias+scale)
PRELU_OP = _register_op(
    "ANT_PRELU_BIAS", (Src0 + C0) * C1 + relu(Src0 + C0) * C2,
    lambda in0, in1, s0, s1, imm2: (in0 + s0) * s1 + np.maximum(in0 + s0, 0) * imm2)
# out = (x*c1 + c0) - y*y     (variance from E[h^2], mu, +eps)
VFUSE_OP = _register_op(
    "ANT_VFUSE", (Src0 * C1 + C0) - Src1 * Src1,
    lambda in0, in1, s0, s1, imm2: (in0 * s1 + s0) - in1 * in1)
# out = y*(c0 - (v*y)*y*c1)   (one rsqrt newton step; v=Src0, y=Src1)
RSQNR_OP = _register_op(
    "ANT_RSQ_NR", Src1 * (C0 - (Src0 * Src1) * Src1 * C1),
    lambda in0, in1, s0, s1, imm2: in1 * (s0 - (in0 * in1) * in1 * s1))

F32 = mybir.dt.float32
BF16 = mybir.dt.bfloat16
FP8 = mybir.dt.float8e4
I32 = mybir.dt.int32
AF = mybir.ActivationFunctionType
OP = mybir.AluOpType
DR = mybir.MatmulPerfMode.DoubleRow

E4NP = ml_dtypes.float8_e4m3
BFNP = ml_dtypes.bfloat16

N_CORES = 8
NPEDS = 8192
NP_CORE = NPEDS // N_CORES      # 1024
HALF = NP_CORE // 2             # 512
H = 256
E = 64
T = 30
EPS = 1e-5
LEAK = 0.01
MAGIC = 0x5F3759DF
KAP = 240.0                     # ones-row magnitude for bias terms
TANH_S_SCALE = 0.88 / float(np.sqrt(4.0 * EPS))

# gate chunk order along stationary free axis: ch-major, (i,f,o,g)
GATE_BASE = {"i": 0, "f": H, "g": 2 * H, "o": 3 * H}
CHUNKS = [(g, ch) for ch in range(2) for g in ("i", "f", "o", "g")]


def _chunk_rows(gate, ch):
    b = GATE_BASE[gate] + ch * 128
    return slice(b, b + 128)


DEBUG_TAPS = False


def _build_program():
    nc = bacc.Bacc(
        "TRN2",
        target_bir_lowering=False,
        debug=False,
        enable_asserts=False,
        num_devices=N_CORES,
    )

    d = {}
    d["LWH"] = nc.dram_tensor("LWH", [128, 8, 2, 128], FP8, kind="ExternalInput")
    d["LWHD"] = nc.dram_tensor("LWHD", [128, 2, 2, 128], FP8, kind="ExternalInput")
    d["LWD"] = nc.dram_tensor("LWD", [66, 8, 2, 128], FP8, kind="ExternalInput")
    d["AS"] = nc.dram_tensor("AS", [128, 2, 32], BF16, kind="ExternalInput")
    d["SQB"] = nc.dram_tensor("SQB", [128, 2, 32], BF16, kind="ExternalInput")
    d["WEMB"] = nc.dram_tensor("WEMB", [1, 64], BF16, kind="ExternalInput")
    d["EMBB"] = nc.dram_tensor("EMBB", [64, 1], F32, kind="ExternalInput")
    d["PB0"] = nc.dram_tensor("PB0", [32, 1], F32, kind="ExternalInput")
    d["PB1"] = nc.dram_tensor("PB1", [32, 1], F32, kind="ExternalInput")
    d["H0"] = nc.dram_tensor("H0", [128, 2, NP_CORE], BF16, kind="ExternalInput")
    d["H08"] = nc.dram_tensor("H08", [128, 2, NP_CORE], FP8, kind="ExternalInput")
    d["C0"] = nc.dram_tensor("C0", [128, 2, NP_CORE], BF16, kind="ExternalInput")
    d["DEC0"] = nc.dram_tensor("DEC0", [66, NP_CORE], FP8, kind="ExternalInput")
    out_t = nc.dram_tensor("OUT", [T, 2, 32, 32], F32, kind="ExternalOutput")
    dbg = {}
    if DEBUG_TAPS:
        for nm, shp in [("dSG", [128, 2, 3, HALF]), ("dTG", [128, 2, HALF]),
                        ("dCS", [128, 2, HALF]), ("dTB", [128, 2, HALF]),
                        ("dH8", [128, 2, HALF]), ("dHSQ", [128, 2, HALF]),
                        ("dST", [32, HALF]), ("dTT", [32, HALF]),
                        ("dV", [32, 16]), ("dR", [32, 16]),
                        ("dDEC", [66, HALF])]:
            dbg[nm] = nc.dram_tensor(nm, shp, F32, kind="ExternalOutput")

    inv_s = None  # patched below once S is known at prep time; use attr
    # activation scale 1/S is a compile-time constant: S is fixed pow2
    # computed from the spec'd weight magnitudes; recompute here must match
    # _prepare_in_maps. We hardcode S=512 (see _prep: weights ~N(0,0.05),
    # max|W| in (0.109, 0.4375] -> S=512).
    S = 512.0
    INV_S = 1.0 / S

    with tile.TileContext(nc) as tc:
        with (
            tc.tile_pool(name="weights", bufs=1) as wp,
            tc.tile_pool(name="state", bufs=1) as sp,
            tc.tile_pool(name="acts", bufs=2) as ap_,
            tc.tile_pool(name="dve", bufs=4) as dp,
            tc.tile_pool(name="tail", bufs=2) as tp,
            tc.tile_pool(name="tappool", bufs=1) as tapp,
            tc.tile_pool(name="pifo", bufs=2, space="PSUM") as pifo,
            tc.tile_pool(name="psm", bufs=2, space="PSUM") as psm,
        ):
            # ---- persistent weights ----
            LWH = [wp.tile([128, 2, 128], FP8, name=f"LWH{c}", tag=f"LWH{c}") for c in range(8)]
            LWHD = [wp.tile([128, 2, 128], FP8, name=f"LWHD{c}", tag=f"LWHD{c}") for c in range(2)]
            LWD = [wp.tile([66, 2, 128], FP8, name=f"LWD{c}", tag=f"LWD{c}") for c in range(8)]
            AS = wp.tile([128, 2, 32], BF16, tag="AS")
            SQB = wp.tile([128, 2, 32], BF16, tag="SQB")
            WEMB = wp.tile([1, 64], BF16, tag="WEMB")
            EMBB = wp.tile([64, 1], F32, tag="EMBB")
            PB0 = wp.tile([32, 1], F32, tag="PB0")
            PB1 = wp.tile([32, 1], F32, tag="PB1")
            IONE = wp.tile([32, 16], I32, tag="IONE")
            IMAG = wp.tile([32, 16], I32, tag="IMAG")
            for c in range(8):
                nc.sync.dma_start(LWH[c][:], d["LWH"].ap()[:, c])
                nc.sync.dma_start(LWD[c][:], d["LWD"].ap()[:, c])
            for c in range(2):
                nc.sync.dma_start(LWHD[c][:], d["LWHD"].ap()[:, c])
            for nm, t_ in [("AS", AS), ("SQB", SQB), ("WEMB", WEMB),
                           ("EMBB", EMBB), ("PB0", PB0), ("PB1", PB1)]:
                nc.sync.dma_start(t_[:], d[nm].ap())
            nc.vector.memset(IONE[:], 1)
            nc.vector.memset(IMAG[:], MAGIC)

            # ---- persistent states [half][parity] ----
            TB = [[sp.tile([128, 2, HALF], BF16, name=f"TB{h}{p}", tag=f"TB{h}{p}")
                   for p in range(2)] for h in range(2)]
            H8 = [[sp.tile([128, 2, HALF], FP8, name=f"H8{h}{p}", tag=f"H8{h}{p}")
                   for p in range(2)] for h in range(2)]
            CS = [[sp.tile([128, 2, HALF], BF16, name=f"CS{h}{p}", tag=f"CS{h}{p}")
                   for p in range(2)] for h in range(2)]
            DEC8 = [[sp.tile([66, HALF], FP8, name=f"DEC{h}{p}", tag=f"DEC{h}{p}")
                     for p in range(2)] for h in range(2)]
            SSB = [sp.tile([32, HALF], BF16, name=f"SSB{h}", tag=f"SSB{h}") for h in range(2)]
            for h in range(2):
                cols = slice(h * HALF, (h + 1) * HALF)
                nc.sync.dma_start(TB[h][0][:], d["H0"].ap()[:, :, cols])
                nc.sync.dma_start(H8[h][0][:], d["H08"].ap()[:, :, cols])
                nc.sync.dma_start(CS[h][0][:], d["C0"].ap()[:, :, cols])
                nc.sync.dma_start(DEC8[h][0][:], d["DEC0"].ap()[:, cols])
                nc.vector.memset(DEC8[h][1][64:66, :], 1.0)
                nc.vector.memset(SSB[h][:], 0.0)

            def tap(nm, src, shp):
                if not DEBUG_TAPS:
                    return
                tt_ = tapp.tile(shp, F32, name=f"tap{nm}", tag=f"tap{nm}")
                nc.scalar.activation(tt_[:], src, AF.Copy)
                nc.sync.dma_start(dbg[nm].ap(), tt_[:])

            def g_mms(h, p):
                """All gate matmuls for one half; returns psum pieces."""
                h8 = H8[h][p][:]
                dec = DEC8[h][p][:].unsqueeze(1).broadcast_to([66, 2, HALF])
                waves = []
                for ch in range(2):
                    wave = pifo.tile([128, 3, HALF], F32, tag="ifo")
                    for gi, gate in enumerate(("i", "f", "o")):
                        c = ch * 4 + {"i": 0, "f": 1, "o": 2}[gate]
                        o_ = wave[:, gi, :]
                        nc.tensor.matmul(o_, LWH[c][:], h8, start=True,
                                         stop=False, perf_mode=DR)
                        nc.tensor.matmul(o_, LWD[c][:], dec, start=False,
                                         stop=True, perf_mode=DR)
                    waves.append(wave)
                gps = []
                for ch in range(2):
                    gp = psm.tile([128, HALF], F32, tag="sm")
                    c = ch * 4 + 3
                    nc.tensor.matmul(gp[:], LWH[c][:], h8, start=True,
                                     stop=False, perf_mode=DR)
                    nc.tensor.matmul(gp[:], LWHD[ch][:], h8, start=False,
                                     stop=False, perf_mode=DR)
                    nc.tensor.matmul(gp[:], LWD[c][:], dec, start=False,
                                     stop=True, perf_mode=DR)
                    gps.append(gp)
                sg = ap_.tile([128, 2, 3, HALF], BF16, name=f"SG{h}",
                              tag=f"SG{h}")
                tg = ap_.tile([128, 2, HALF], BF16, name=f"TG{h}",
                              tag=f"TG{h}")
                return {"waves": waves, "gps": gps, "sg": sg, "tg": tg}

            def act_sig(h, G, ch):
                nc.scalar.activation(G["sg"][:, ch, :, :], G["waves"][ch][:],
                                     AF.Sigmoid, scale=INV_S)

            def act_tg(h, G, ch):
                nc.scalar.activation(G["tg"][:, ch, :], G["gps"][ch][:],
                                     AF.Tanh, scale=INV_S)

            def cell_dve(h, p, q, G):
                sg, tg = G["sg"], G["tg"]
                m1 = dp.tile([128, 2, HALF], BF16, tag="m1")
                nc.vector.tensor_tensor(m1[:], sg[:, :, 1, :], CS[h][p][:],
                                        OP.mult)
                m2 = dp.tile([128, 2, HALF], BF16, tag="m2")
                nc.vector.tensor_tensor(m2[:], sg[:, :, 0, :], tg[:], OP.mult)
                nc.vector.tensor_tensor(CS[h][q][:], m1[:], m2[:], OP.add)

            def act_tc(h, q, G):
                tc_ = ap_.tile([128, 2, HALF], BF16, name=f"TC{h}",
                               tag=f"TC{h}")
                nc.scalar.activation(tc_[:], CS[h][q][:], AF.Tanh)
                G["tc"] = tc_

            def hn_dve(h, q, G):
                sg_o = G["sg"][:, :, 2, :]
                tc_ = G["tc"]
                nc.vector.tensor_tensor(TB[h][q][:], sg_o, tc_[:], OP.mult)
                # fp8 h copy on Pool (needed only by next step's gate mms)
                nc.gpsimd.tensor_tensor(H8[h][q][:], sg_o, tc_[:], OP.mult)

            def act_sq(h, q):
                hsq = dp.tile([128, 2, HALF], BF16, name=f"HSQ{h}",
                              tag=f"HSQ{h}")
                nc.scalar.activation(hsq[:], TB[h][q][:], AF.Square)
                return hsq

            def stats(h, q, hsq):
                stt = psm.tile([128, HALF], F32, tag="sm")
                st = stt[0:32, :]
                nc.tensor.matmul(st, AS[:, 0, :], TB[h][q][:, 0, :],
                                 start=True, stop=False)
                nc.tensor.matmul(st, AS[:, 1, :], TB[h][q][:, 1, :],
                                 start=False, stop=False)
                nc.tensor.matmul(st, SQB[:, 0, :], hsq[:, 0, :],
                                 start=False, stop=False)
                nc.tensor.matmul(st, SQB[:, 1, :], hsq[:, 1, :],
                                 start=False, stop=True)
                return st

            def tail(t_, h, st, dotap=False):
                if dotap:
                    tap("dST", st, [32, HALF])
                tailT = tp.tile([32, HALF], F32, tag="tailT")
                nc.vector.transpose(tailT[:], st)
                if dotap:
                    tap("dTT", tailT[:], [32, HALF])
                c_num0 = tailT[:, 0::32]
                c_num1 = tailT[:, 1::32]
                c_mu = tailT[:, 2::32]
                c_eh2 = tailT[:, 3::32]
                V = dp.tile([32, 16], F32, tag="V")
                nc.vector._custom_dve(VFUSE_OP, out=V[:], in0=c_eh2,
                                      in1=c_mu, s0=EPS, s1=1.0)
                sh = dp.tile([32, 16], I32, tag="sh")
                nc.vector.tensor_tensor(sh[:], V[:].bitcast(I32), IONE[:],
                                        OP.arith_shift_right)
                y = dp.tile([32, 16], F32, tag="y")
                nc.vector.tensor_tensor(y[:].bitcast(I32), IMAG[:], sh[:],
                                        OP.subtract)
                r = dp.tile([32, 16], F32, tag="r")
                nc.vector._custom_dve(RSQNR_OP, out=r[:], in0=V[:], in1=y[:],
                                      s0=1.5, s1=0.5)
                if dotap:
                    tap("dV", V[:], [32, 16])
                    tap("dR", r[:], [32, 16])
                z0 = dp.tile([32, 16], F32, tag="z0")
                nc.vector.tensor_tensor(z0[:], c_num0, r[:], OP.mult)
                z1 = dp.tile([32, 16], F32, tag="z1")
                nc.vector.tensor_tensor(z1[:], c_num1, r[:], OP.mult)
                ts = tp.tile([32, 32], F32, tag="TS")
                nc.scalar.activation(ts[:, 0:16], z0[:], AF.Sigmoid,
                                     bias=PB0[:])
                nc.scalar.activation(ts[:, 16:32], z1[:], AF.Sigmoid,
                                     bias=PB1[:])
                e = dp.tile([32, 16], F32, tag="e")
                nc.vector.tensor_tensor(e[:], ts[:, 0:16], ts[:, 16:32],
                                        OP.subtract)
                nc.scalar.activation(SSB[h][:, 0::32], e[:], AF.Tanh,
                                     scale=TANH_S_SCALE)
                nc.sync.dma_start(out_t.ap()[t_][h], ts[:])
                sB = tp.tile([32, HALF], BF16, tag="sB")
                nc.vector.transpose(sB[:], SSB[h][:])
                return sB

            def embed(h, q, sB, dotap=False):
                pet = psm.tile([128, HALF], F32, tag="sm")
                pe = pet[0:64, :]
                nc.tensor.matmul(pe, WEMB[:], sB[0:1, :], start=True,
                                 stop=True)
                nc.vector._custom_dve(PRELU_OP, out=DEC8[h][q][0:64, :],
                                      in0=pe, s0=EMBB[:], s1=LEAK, imm2=1.0 - LEAK)
                if dotap:
                    tap("dDEC", DEC8[h][q][:], [66, HALF])

            sB_prev = None
            for t_ in range(T):
                p, q = t_ % 2, (t_ + 1) % 2
                dt0 = DEBUG_TAPS and t_ == 0
                if sB_prev is not None:
                    # embed mms for step t-1 run first: inputs long ready,
                    # so the PE never stalls on the tail chain.
                    embed(0, p, sB_prev[0], dotap=dt0)
                    embed(1, p, sB_prev[1])
                G0 = g_mms(0, p)
                G1 = g_mms(1, p)
                act_sig(0, G0, 0)
                act_sig(0, G0, 1)
                act_tg(0, G0, 0)
                act_tg(0, G0, 1)
                cell_dve(0, p, q, G0)
                act_sig(1, G1, 0)
                act_tc(0, q, G0)
                act_sig(1, G1, 1)
                hn_dve(0, q, G0)
                hsq0 = act_sq(0, q)
                act_tg(1, G1, 0)
                act_tg(1, G1, 1)
                cell_dve(1, p, q, G1)
                act_tc(1, q, G1)
                hn_dve(1, q, G1)
                hsq1 = act_sq(1, q)
                st0 = stats(0, q, hsq0)
                st1 = stats(1, q, hsq1)
                sB0 = tail(t_, 0, st0, dotap=dt0)
                sB1 = tail(t_, 1, st1)
                sB_prev = (sB0, sB1)

    nc.compile()
    return nc


_NC_CACHE = None


def _get_program():
    global _NC_CACHE
    if _NC_CACHE is None:
        _NC_CACHE = _build_program()
    return _NC_CACHE


def _prepare_in_maps(inputs):
    f32 = np.float32
    inp = {k: np.asarray(v, f32) for k, v in inputs.items()}
    W_ih, W_hh = inp["W_ih"], inp["W_hh"]
    bias = (inp["b_ih"] + inp["b_hh"]).astype(f32)

    S = 2.0 ** np.floor(np.log2(224.0 / max(np.abs(W_hh).max(),
                                            np.abs(W_ih).max())))
    assert S == 512.0, f"S={S} changed; update INV_S in _build_program"

    def q8(x):
        return np.asarray(x, E4NP).astype(f32)

    Whh8 = q8(W_hh * S)
    Wih8 = q8(W_ih * S)
    dWhh8 = q8(W_hh * S - Whh8)
    dWih8 = q8(W_ih * S - Wih8)
    bhi = q8(bias * S)
    blo = q8(bias * S - bhi)
    r1 = bias * S - (bhi + blo)
    bhi2 = q8(r1)
    blo2 = q8(r1 - bhi2)

    LWH = np.zeros((128, 8, 2, 128), f32)
    LWHD = np.zeros((128, 2, 2, 128), f32)
    LWD = np.zeros((66, 8, 2, 128), f32)
    for ci, (gate, ch) in enumerate(CHUNKS):
        rows = _chunk_rows(gate, ch)
        for j in range(2):
            LWH[:, ci, j, :] = Whh8[rows, 128 * j:128 * (j + 1)].T
            LWD[0:64, ci, j, :] = (Wih8 if j == 0 else dWih8)[rows].T
        LWD[64, ci, 0, :] = bhi[rows]
        LWD[64, ci, 1, :] = blo[rows]
        LWD[65, ci, 0, :] = bhi2[rows]
        LWD[65, ci, 1, :] = blo2[rows]
    for ch in range(2):
        rows = _chunk_rows("g", ch)
        for j in range(2):
            LWHD[:, ch, j, :] = dWhh8[rows, 128 * j:128 * (j + 1)].T

    emb_W, emb_b = inp["emb_W"], inp["emb_b"]
    g1, b1 = inp["ln1_g"], inp["ln1_b"]
    w_emb = (g1[0] * emb_W[:, 0] - g1[1] * emb_W[:, 1]).astype(f32)
    emb_bp = (emb_b + b1[0] * emb_W[:, 0] + b1[1] * emb_W[:, 1]).astype(f32)
    WEMB = w_emb[None, :].astype(f32)

    pos_W, pos_b = inp["pos_W"], inp["pos_b"]
    g2, b2 = inp["ln2_g"], inp["ln2_b"]
    posWp = (pos_W * g2[None, :]).astype(f32)
    pos_bp = (pos_b + b2 @ pos_W.T).astype(f32)
    A = posWp - posWp.sum(1)[:, None] / H

    ASm = np.zeros((128, 2, 32), f32)
    for ch in range(2):
        ASm[:, ch, 0] = A[0, 128 * ch:128 * (ch + 1)]
        ASm[:, ch, 1] = A[1, 128 * ch:128 * (ch + 1)]
        ASm[:, ch, 2] = 1.0 / H
    SQB = np.zeros((128, 2, 32), f32)
    SQB[:, :, 3] = 1.0 / H

    lpr = inp["last_pos_rel"]
    e0 = lpr[:, 0] - lpr[:, 1]
    s0 = e0 / np.sqrt(e0 * e0 + 4 * EPS)
    z = s0[:, None] * w_emb[None, :] + emb_bp[None, :]
    dec0 = np.where(z > 0, z, LEAK * z).astype(f32)       # [N, 64]
    DEC0 = np.ones((66, NPEDS), f32)
    DEC0[0:64] = dec0.T

    h0 = inp["h0"][0]                                      # [N, 256]
    c0 = inp["c0"][0]
    h0p = np.ascontiguousarray(h0.T.reshape(2, 128, NPEDS).transpose(1, 0, 2))
    c0p = np.ascontiguousarray(c0.T.reshape(2, 128, NPEDS).transpose(1, 0, 2))

    rep = {
        "LWH": LWH.astype(E4NP), "LWHD": LWHD.astype(E4NP),
        "LWD": LWD.astype(E4NP),
        "AS": ASm.astype(BFNP), "SQB": SQB.astype(BFNP),
        "WEMB": WEMB.astype(BFNP),
        "EMBB": np.ascontiguousarray(emb_bp[:, None]),
        "PB0": np.full((32, 1), pos_bp[0], f32),
        "PB1": np.full((32, 1), pos_bp[1], f32),
    }
    in_maps = []
    for c in range(N_CORES):
        cols = slice(c * NP_CORE, (c + 1) * NP_CORE)
        m = dict(rep)
        m["H0"] = np.ascontiguousarray(h0p[:, :, cols]).astype(BFNP)
        m["H08"] = np.ascontiguousarray(h0p[:, :, cols]).astype(E4NP)
        m["C0"] = np.ascontiguousarray(c0p[:, :, cols]).astype(BFNP)
        m["DEC0"] = np.ascontiguousarray(DEC0[:, cols]).astype(E4NP)
        in_maps.append(m)
    return in_maps


def _unscramble(raw):
    """raw: [T, 2, 32, 32] per core -> [T, NP_CORE, 2]."""
    o = raw.reshape(T, 2, 32, 2, 16)          # t, half, p, j, k
    o = o.transpose(0, 1, 4, 2, 3)            # t, half, k, p, j
    return o.reshape(T, NP_CORE, 2)


def run_on_hw(inputs, trace=False, **kwargs):
    nc = _get_program()
    in_maps = _prepare_in_maps(inputs)
    old_m = nc.m
    nc.m = get_hw_module(nc.m)
    try:
        res = bass_utils.run_bass_kernel_spmd(
            nc, in_maps, core_ids=list(range(N_CORES)), trace=trace, **kwargs)
    finally:
        nc.m = old_m
    out = np.concatenate([_unscramble(np.asarray(r["OUT"], np.float32))
                          for r in res.results], axis=1)
    return out.astype(np.float32), res


def kernel(**inputs) -> np.ndarray:
    out, _ = run_on_hw(inputs, trace=False)
    return out
